# revision 18
# baseline (speedup 1.0000x reference)
"""Grouped-experts SwiGLU kernel for 8 Trainium2 NeuronCores.

Problem: x[E,T,D], w1[E,D,H], w2[E,H,D], w3[E,D,H] with E=8, T=1024,
D=1024, H=2048.  out_e = (silu(x_e @ w1_e) * (x_e @ w3_e)) @ w2_e.

Sharding: expert-parallel, one expert per NeuronCore (E == n_cores == 8).
Each core runs an identical Bass program on its expert's slices; no
collectives are needed and the full output is just the stack of the
per-core outputs.

All matmul inputs are staged host-side into bf16 (end-to-end rel err
~3.6e-3 vs the 2e-2 budget; PSUM accumulation stays fp32):
  xt  [D, T]        x transposed (partition dim = D, ready for the PE)
  w1r [H, ND*128]   w1 reordered so tile hh is one contiguous 256KB block
                    with layout [p=d%128, dd, h-col]
  w3r               same as w1r
  w2r [H, D]        natural (row block hh is the stage-B rhs)

Per-core schedule (bf16 matmuls run 1 col/cycle at N=512):
  warm-up: a dozen dummy matmuls so the HAM clock gate un-throttles to
           2.4 GHz right as the first real matmul arrives.
  Stage A: for each of the 16 H-tiles, stream w1r/w3r slices (scalar HW
           DMA queue), compute gT/upT = w^T @ x^T in PSUM accumulating
           over the 8 D-chunks, silu on ScalarE, multiply on VectorE ->
           resident hT bf16 (partition = H).
  Stage B: w2 fully resident (prefetched on the sync queue behind xt),
           out = h @ w2 accumulating over the 16 H-chunks; PSUM
           evacuated via ScalarE/VectorE copies, DMA out in natural
           [T,D] fp32 layout.

xt / w2 are loaded with two bulk multi-descriptor DMAs each (DMA issue
instructions cost ~600ns on the issuing engine, so fewer is better),
and the big resident operands are single tiles (Tile emits a release +
barrier chain per tile buffer at context exit; fewer buffers = shorter
epilogue).
"""

import sys

if "/opt/trn_rl_repo" not in sys.path:
    sys.path.insert(0, "/opt/trn_rl_repo")

import numpy as np
import ml_dtypes

E, T, D, H = 8, 1024, 1024, 2048
P = 128
NT, ND, NH = T // P, D // P, H // P
TC = 512  # stage-A moving (token) chunk
NTC = T // TC
HG = 2  # h-tiles per weight-stream DMA group
NHG = NH // HG
N_WARM = 10
BF16 = ml_dtypes.bfloat16


def build_program(reps: int = 1):
    """Build the per-core Bass program. reps>1 repeats the whole compute
    body (for wall-clock slope timing); the result is identical."""
    import concourse.bacc as bacc
    import concourse.mybir as mybir
    from concourse import tile

    f32 = mybir.dt.float32
    bf16 = mybir.dt.bfloat16
    SILU = mybir.ActivationFunctionType.Silu

    nc = bacc.Bacc("TRN2", target_bir_lowering=False, debug=False)
    xt_d = nc.declare_dram_parameter("xt", [D, T], bf16, isOutput=False)
    w1_d = nc.declare_dram_parameter("w1r", [H, ND * P], bf16, isOutput=False)
    w2_d = nc.declare_dram_parameter("w2r", [H, D], bf16, isOutput=False)
    w3_d = nc.declare_dram_parameter("w3r", [H, ND * P], bf16, isOutput=False)
    out_d = nc.declare_dram_parameter("out", [T, D], f32, isOutput=True)

    xt_v = xt_d[:].rearrange("(dd p) t -> p dd t", p=P)  # [128, ND, T]
    w2_v = w2_d[:].rearrange("(hh p) d -> p hh d", p=P)  # [128, NH, D]

    with tile.TileContext(nc) as tc:
        with (
            tc.tile_pool(name="warm", bufs=1) as warm_pool,
            tc.tile_pool(name="xT", bufs=1) as xT_pool,
            tc.tile_pool(name="hT", bufs=1) as hT_pool,
            tc.tile_pool(name="w2s", bufs=1) as w2_pool,
            tc.tile_pool(name="wA", bufs=3) as wA_pool,
            tc.tile_pool(name="sg", bufs=3) as sg_pool,
            tc.tile_pool(name="ob", bufs=3) as ob_pool,
            tc.tile_pool(name="ps", bufs=8, space="PSUM") as ps_pool,
        ):
            # Scratch operands for engine warm-up (contents irrelevant).
            wu = warm_pool.tile([P, 4 * P], bf16, name="wu", tag="wu")
            nc.gpsimd.memset(wu[:], 0.0)
            sc = warm_pool.tile([P, 64], f32, name="sc", tag="sc")
            nc.gpsimd.memset(sc[:], 0.0)

            for rep in range(reps):
                # ---- input DMAs -----------------------------------------
                # The two HW DMA queues (sync, scalar) share the 16 DMA
                # engines, so the critical first chunk (xt tokens 0:512 +
                # the first w1 group) is balanced across both rings; w2
                # follows on the sync ring and streams during stage A.
                xT = xT_pool.tile([P, ND, T], bf16, name="xT", tag="xT")
                # sync ring: the three xt pieces in consumption order; the
                # 256-token first piece minimizes stage A's critical bytes.
                for a, b in ((0, 256), (256, TC), (TC, T)):
                    nc.sync.dma_start(
                        out=xT[:, :, a:b], in_=xt_v[:, :, a:b]
                    )
                w2s = w2_pool.tile([P, NH, D], bf16, name="w2s", tag="w2s")
                for g in range(2):
                    hs = slice(g * NH // 2, (g + 1) * NH // 2)
                    nc.sync.dma_start(out=w2s[:, hs, :], in_=w2_v[:, hs, :])

                # ---- Stage A: hT = silu(w1^T x^T) * (w3^T x^T) ----------
                hT = hT_pool.tile([P, NH, T], bf16, name="hT", tag="hT")
                w1_v = w1_d[:].rearrange("(hh p) (dd c) -> p hh dd c", p=P, c=P)
                w3_v = w3_d[:].rearrange("(hh p) (dd c) -> p hh dd c", p=P, c=P)

                def load_pair(hg, split_first=False):
                    hgs = slice(hg * HG, (hg + 1) * HG)
                    w1s = wA_pool.tile([P, HG, ND, P], bf16, name="w1s", tag="w1s")
                    w3s = wA_pool.tile([P, HG, ND, P], bf16, name="w3s", tag="w3s")
                    if not split_first:
                        nc.scalar.dma_start(out=w1s[:], in_=w1_v[:, hgs, :, :])
                        nc.scalar.dma_start(out=w3s[:], in_=w3_v[:, hgs, :, :])
                        return w1s, w3s
                    # Head of the kernel: per-hh pieces so the first
                    # matmuls gate on as few critical bytes as possible.
                    nc.scalar.dma_start(
                        out=w1s[:, 0:1, :, :], in_=w1_v[:, 0:1, :, :]
                    )
                    if rep == 0:
                        # Forces the SILU act-table load onto the idle head
                        # of the Scalar queue (it is auto-inserted before
                        # the first ACTIVATE in program order; left alone
                        # it lands mid stage A and stalls PSUM drains).
                        nc.scalar.activation(sc[:], sc[:], SILU)
                    nc.scalar.dma_start(
                        out=w3s[:, 0:1, :, :], in_=w3_v[:, 0:1, :, :]
                    )
                    nc.scalar.dma_start(
                        out=w1s[:, 1:2, :, :], in_=w1_v[:, 1:2, :, :]
                    )
                    nc.scalar.dma_start(
                        out=w3s[:, 1:2, :, :], in_=w3_v[:, 1:2, :, :]
                    )
                    return w1s, w3s

                def mm_chunk(w1s, w3s, j, hh, a, b):
                    tok = slice(a, b)
                    n = b - a
                    g_ps = ps_pool.tile([P, n], f32, name="g_ps", tag="ps")
                    u_ps = ps_pool.tile([P, n], f32, name="u_ps", tag="ps")
                    for dd in range(ND):
                        nc.tensor.matmul(
                            g_ps[:],
                            w1s[:, j, dd, :],
                            xT[:, dd, tok],
                            start=(dd == 0),
                            stop=(dd == ND - 1),
                        )
                    for dd in range(ND):
                        nc.tensor.matmul(
                            u_ps[:],
                            w3s[:, j, dd, :],
                            xT[:, dd, tok],
                            start=(dd == 0),
                            stop=(dd == ND - 1),
                        )
                    sg = sg_pool.tile([P, n], f32, name="sg", tag="sg")
                    nc.scalar.activation(sg[:], g_ps[:], SILU)
                    nc.vector.tensor_mul(hT[:, hh, tok], sg[:], u_ps[:])

                # ---- PE warm-up: dummy matmuls so the HAM clock gate
                # un-throttles before the first real matmul.
                if rep == 0:
                    wu_ps = ps_pool.tile([P, 4 * P], f32, name="wu_ps", tag="ps")
                    for _ in range(N_WARM):
                        nc.tensor.matmul(
                            wu_ps[:], wu[:, :P], wu[:], start=True, stop=True
                        )

                # First group chases the xt pieces: both hh tiles consume
                # each 256-token piece before the next piece is needed,
                # buying the DMA stream compute-time to stay ahead.
                w1s0, w3s0 = load_pair(0, split_first=True)
                mm_chunk(w1s0, w3s0, 0, 0, 0, 256)
                mm_chunk(w1s0, w3s0, 1, 1, 0, 256)
                mm_chunk(w1s0, w3s0, 0, 0, 256, TC)
                mm_chunk(w1s0, w3s0, 1, 1, 256, TC)
                mm_chunk(w1s0, w3s0, 0, 0, TC, T)
                mm_chunk(w1s0, w3s0, 1, 1, TC, T)
                for hg in range(1, NHG):
                    w1s, w3s = load_pair(hg)
                    for j in range(HG):
                        hh = hg * HG + j
                        for c in range(NTC):
                            mm_chunk(w1s, w3s, j, hh, c * TC, (c + 1) * TC)

                # ---- Stage B: out = h @ w2 ------------------------------
                for t in range(NT):
                    for dc in range(2):
                        if t == NT - 1 and dc == 1:
                            # Last block in 256-col pieces: halves the
                            # post-final-matmul copy+DMA drain.
                            chunks = [(TC, TC + 256), (TC + 256, T)]
                        else:
                            chunks = [(dc * TC, (dc + 1) * TC)]
                        for a, b in chunks:
                            o_ps = ps_pool.tile([P, b - a], f32, name="o_ps", tag="ps")
                            for hh in range(NH):
                                nc.tensor.matmul(
                                    o_ps[:],
                                    hT[:, hh, t * P : (t + 1) * P],
                                    w2s[:, hh, a:b],
                                    start=(hh == 0),
                                    stop=(hh == NH - 1),
                                )
                            ob = ob_pool.tile([P, b - a], f32, name="ob", tag="ob")
                            nc.vector.tensor_copy(ob[:], o_ps[:])
                            nc.scalar.dma_start(
                                out=out_d[t * P : (t + 1) * P, a:b], in_=ob[:]
                            )

    nc.compile()
    return nc


_program_cache = {}


def _get_program(reps: int = 1):
    if reps not in _program_cache:
        _program_cache[reps] = build_program(reps)
    return _program_cache[reps]


def stage_inputs(x_e, w1_e, w2_e, w3_e):
    """Host-side staging of one expert's inputs into the device layouts."""
    xt = np.ascontiguousarray(x_e.T).astype(BF16)
    w1r = (
        w1_e.reshape(ND, P, NH, P)
        .transpose(2, 1, 0, 3)
        .astype(BF16)
        .reshape(H, ND * P)
    )
    w3r = (
        w3_e.reshape(ND, P, NH, P)
        .transpose(2, 1, 0, 3)
        .astype(BF16)
        .reshape(H, ND * P)
    )
    w2r = w2_e.astype(BF16)
    return {"xt": xt, "w1r": w1r, "w2r": w2r, "w3r": w3r}


def kernel(x, w1, w2, w3):
    from concourse.bass_utils import run_bass_kernel_spmd

    x = np.asarray(x, dtype=np.float32)
    w1 = np.asarray(w1, dtype=np.float32)
    w2 = np.asarray(w2, dtype=np.float32)
    w3 = np.asarray(w3, dtype=np.float32)

    nc = _get_program()
    in_maps = [stage_inputs(x[e], w1[e], w2[e], w3[e]) for e in range(E)]
    res = run_bass_kernel_spmd(nc, in_maps, list(range(E)))
    out = np.stack([res.results[e]["out"] for e in range(E)], axis=0)
    return out.astype(np.float32)


# revision 19
# speedup vs baseline: 1.0139x; 1.0139x over previous
"""Grouped-experts SwiGLU kernel for 8 Trainium2 NeuronCores.

Problem: x[E,T,D], w1[E,D,H], w2[E,H,D], w3[E,D,H] with E=8, T=1024,
D=1024, H=2048.  out_e = (silu(x_e @ w1_e) * (x_e @ w3_e)) @ w2_e.

Sharding: expert-parallel, one expert per NeuronCore (E == n_cores == 8).
Each core runs an identical Bass program on its expert's slices; no
collectives are needed and the full output is just the stack of the
per-core outputs.

All matmul inputs are staged host-side into bf16 (end-to-end rel err
~3.6e-3 vs the 2e-2 budget; PSUM accumulation stays fp32):
  xt  [D, T]        x transposed (partition dim = D, ready for the PE)
  w1r [H, ND*128]   w1 reordered so tile hh is one contiguous 256KB block
                    with layout [p=d%128, dd, h-col]
  w3r               same as w1r
  w2r [H, D]        natural (row block hh is the stage-B rhs)

Per-core schedule (bf16 matmuls run 1 col/cycle at N=512):
  warm-up: a dozen dummy matmuls so the HAM clock gate un-throttles to
           2.4 GHz right as the first real matmul arrives.
  Stage A: for each of the 16 H-tiles, stream w1r/w3r slices (scalar HW
           DMA queue), compute gT/upT = w^T @ x^T in PSUM accumulating
           over the 8 D-chunks, silu on ScalarE, multiply on VectorE ->
           resident hT bf16 (partition = H).
  Stage B: w2 fully resident (prefetched on the sync queue behind xt),
           out = h @ w2 accumulating over the 16 H-chunks; PSUM
           evacuated via ScalarE/VectorE copies, DMA out in natural
           [T,D] fp32 layout.

xt / w2 are loaded with two bulk multi-descriptor DMAs each (DMA issue
instructions cost ~600ns on the issuing engine, so fewer is better),
and the big resident operands are single tiles (Tile emits a release +
barrier chain per tile buffer at context exit; fewer buffers = shorter
epilogue).
"""

import sys

if "/opt/trn_rl_repo" not in sys.path:
    sys.path.insert(0, "/opt/trn_rl_repo")

import numpy as np
import ml_dtypes

E, T, D, H = 8, 1024, 1024, 2048
P = 128
NT, ND, NH = T // P, D // P, H // P
TC = 512  # stage-A moving (token) chunk
NTC = T // TC
HG = 2  # h-tiles per weight-stream DMA group
NHG = NH // HG
N_WARM = 10
BF16 = ml_dtypes.bfloat16


def build_program(reps: int = 1):
    """Build the per-core Bass program. reps>1 repeats the whole compute
    body (for wall-clock slope timing); the result is identical."""
    import concourse.bacc as bacc
    import concourse.mybir as mybir
    from concourse import tile

    f32 = mybir.dt.float32
    bf16 = mybir.dt.bfloat16
    SILU = mybir.ActivationFunctionType.Silu

    nc = bacc.Bacc("TRN2", target_bir_lowering=False, debug=False)
    xt_d = nc.declare_dram_parameter("xt", [D, T], bf16, isOutput=False)
    w1_d = nc.declare_dram_parameter("w1r", [H, ND * P], bf16, isOutput=False)
    w2_d = nc.declare_dram_parameter("w2r", [H, D], bf16, isOutput=False)
    w3_d = nc.declare_dram_parameter("w3r", [H, ND * P], bf16, isOutput=False)
    out_d = nc.declare_dram_parameter("out", [T, D], f32, isOutput=True)

    xt_v = xt_d[:].rearrange("(dd p) t -> p dd t", p=P)  # [128, ND, T]
    w2_v = w2_d[:].rearrange("(hh p) d -> p hh d", p=P)  # [128, NH, D]

    with tile.TileContext(nc) as tc:
        with (
            tc.tile_pool(name="warm", bufs=1) as warm_pool,
            tc.tile_pool(name="xT", bufs=1) as xT_pool,
            tc.tile_pool(name="hT", bufs=1) as hT_pool,
            tc.tile_pool(name="w2s", bufs=1) as w2_pool,
            tc.tile_pool(name="wA", bufs=3) as wA_pool,
            tc.tile_pool(name="sg", bufs=3) as sg_pool,
            tc.tile_pool(name="ob", bufs=3) as ob_pool,
            tc.tile_pool(name="ps", bufs=8, space="PSUM") as ps_pool,
        ):
            # Scratch operands for engine warm-up (contents irrelevant).
            wu = warm_pool.tile([P, 4 * P], bf16, name="wu", tag="wu")
            nc.gpsimd.memset(wu[:], 0.0)
            sc = warm_pool.tile([P, 64], f32, name="sc", tag="sc")
            nc.gpsimd.memset(sc[:], 0.0)

            for rep in range(reps):
                # ---- input DMAs -----------------------------------------
                # The two HW DMA queues (sync, scalar) share the 16 DMA
                # engines, so the critical first chunk (xt tokens 0:512 +
                # the first w1 group) is balanced across both rings; w2
                # follows on the sync ring and streams during stage A.
                xT = xT_pool.tile([P, ND, T], bf16, name="xT", tag="xT")
                # sync ring: the three xt pieces in consumption order; the
                # 256-token first piece minimizes stage A's critical bytes.
                for a, b in ((0, 256), (256, TC), (TC, T)):
                    nc.sync.dma_start(
                        out=xT[:, :, a:b], in_=xt_v[:, :, a:b]
                    )
                w2s = w2_pool.tile([P, NH, D], bf16, name="w2s", tag="w2s")
                for g in range(2):
                    hs = slice(g * NH // 2, (g + 1) * NH // 2)
                    nc.sync.dma_start(out=w2s[:, hs, :], in_=w2_v[:, hs, :])

                # ---- Stage A: hT = silu(w1^T x^T) * (w3^T x^T) ----------
                hT = hT_pool.tile([P, NH, T], bf16, name="hT", tag="hT")
                w1_v = w1_d[:].rearrange("(hh p) (dd c) -> p hh dd c", p=P, c=P)
                w3_v = w3_d[:].rearrange("(hh p) (dd c) -> p hh dd c", p=P, c=P)

                def load_pair(hg, split_first=False):
                    hgs = slice(hg * HG, (hg + 1) * HG)
                    w1s = wA_pool.tile([P, HG, ND, P], bf16, name="w1s", tag="w1s")
                    w3s = wA_pool.tile([P, HG, ND, P], bf16, name="w3s", tag="w3s")
                    if not split_first:
                        nc.scalar.dma_start(out=w1s[:], in_=w1_v[:, hgs, :, :])
                        nc.scalar.dma_start(out=w3s[:], in_=w3_v[:, hgs, :, :])
                        return w1s, w3s
                    # Head of the kernel: per-hh pieces so the first
                    # matmuls gate on as few critical bytes as possible.
                    nc.scalar.dma_start(
                        out=w1s[:, 0:1, :, :], in_=w1_v[:, 0:1, :, :]
                    )
                    if rep == 0:
                        # Forces the SILU act-table load onto the idle head
                        # of the Scalar queue (it is auto-inserted before
                        # the first ACTIVATE in program order; left alone
                        # it lands mid stage A and stalls PSUM drains).
                        nc.scalar.activation(sc[:], sc[:], SILU)
                    nc.scalar.dma_start(
                        out=w3s[:, 0:1, :, :], in_=w3_v[:, 0:1, :, :]
                    )
                    nc.scalar.dma_start(
                        out=w1s[:, 1:2, :, :], in_=w1_v[:, 1:2, :, :]
                    )
                    nc.scalar.dma_start(
                        out=w3s[:, 1:2, :, :], in_=w3_v[:, 1:2, :, :]
                    )
                    return w1s, w3s

                def mm_chunk(w1s, w3s, j, hh, a, b):
                    tok = slice(a, b)
                    n = b - a
                    g_ps = ps_pool.tile([P, n], f32, name="g_ps", tag="ps")
                    u_ps = ps_pool.tile([P, n], f32, name="u_ps", tag="ps")
                    for dd in range(ND):
                        nc.tensor.matmul(
                            g_ps[:],
                            w1s[:, j, dd, :],
                            xT[:, dd, tok],
                            start=(dd == 0),
                            stop=(dd == ND - 1),
                        )
                    for dd in range(ND):
                        nc.tensor.matmul(
                            u_ps[:],
                            w3s[:, j, dd, :],
                            xT[:, dd, tok],
                            start=(dd == 0),
                            stop=(dd == ND - 1),
                        )
                    sg = sg_pool.tile([P, n], f32, name="sg", tag="sg")
                    nc.scalar.activation(sg[:], g_ps[:], SILU)
                    nc.vector.tensor_mul(hT[:, hh, tok], sg[:], u_ps[:])

                # ---- PE warm-up: dummy matmuls so the HAM clock gate
                # un-throttles before the first real matmul.
                if rep == 0:
                    wu_ps = ps_pool.tile([P, 4 * P], f32, name="wu_ps", tag="ps")
                    for _ in range(N_WARM):
                        nc.tensor.matmul(
                            wu_ps[:], wu[:, :P], wu[:], start=True, stop=True
                        )

                # First group chases the xt pieces: both hh tiles consume
                # each 256-token piece before the next piece is needed,
                # buying the DMA stream compute-time to stay ahead.
                w1s0, w3s0 = load_pair(0, split_first=True)
                mm_chunk(w1s0, w3s0, 0, 0, 0, 256)
                mm_chunk(w1s0, w3s0, 1, 1, 0, 256)
                mm_chunk(w1s0, w3s0, 0, 0, 256, TC)
                mm_chunk(w1s0, w3s0, 1, 1, 256, TC)
                mm_chunk(w1s0, w3s0, 0, 0, TC, T)
                mm_chunk(w1s0, w3s0, 1, 1, TC, T)
                for hg in range(1, NHG):
                    w1s, w3s = load_pair(hg)
                    for j in range(HG):
                        hh = hg * HG + j
                        for c in range(NTC):
                            mm_chunk(w1s, w3s, j, hh, c * TC, (c + 1) * TC)

                # ---- Stage B: out = h @ w2 ------------------------------
                for t in range(NT):
                    for dc in range(2):
                        if t == NT - 1 and dc == 1:
                            # Last block in 256-col pieces: halves the
                            # post-final-matmul copy+DMA drain.
                            chunks = [(TC, TC + 256), (TC + 256, T)]
                        else:
                            chunks = [(dc * TC, (dc + 1) * TC)]
                        for a, b in chunks:
                            o_ps = ps_pool.tile([P, b - a], f32, name="o_ps", tag="ps")
                            for hh in range(NH):
                                nc.tensor.matmul(
                                    o_ps[:],
                                    hT[:, hh, t * P : (t + 1) * P],
                                    w2s[:, hh, a:b],
                                    start=(hh == 0),
                                    stop=(hh == NH - 1),
                                )
                            ob = ob_pool.tile([P, b - a], f32, name="ob", tag="ob")
                            nc.vector.tensor_copy(ob[:], o_ps[:])
                            nc.sync.dma_start(
                                out=out_d[t * P : (t + 1) * P, a:b], in_=ob[:]
                            )

    nc.compile()
    return nc


_program_cache = {}


def _get_program(reps: int = 1):
    if reps not in _program_cache:
        _program_cache[reps] = build_program(reps)
    return _program_cache[reps]


def stage_inputs(x_e, w1_e, w2_e, w3_e):
    """Host-side staging of one expert's inputs into the device layouts."""
    xt = np.ascontiguousarray(x_e.T).astype(BF16)
    w1r = (
        w1_e.reshape(ND, P, NH, P)
        .transpose(2, 1, 0, 3)
        .astype(BF16)
        .reshape(H, ND * P)
    )
    w3r = (
        w3_e.reshape(ND, P, NH, P)
        .transpose(2, 1, 0, 3)
        .astype(BF16)
        .reshape(H, ND * P)
    )
    w2r = w2_e.astype(BF16)
    return {"xt": xt, "w1r": w1r, "w2r": w2r, "w3r": w3r}


def kernel(x, w1, w2, w3):
    from concourse.bass_utils import run_bass_kernel_spmd

    x = np.asarray(x, dtype=np.float32)
    w1 = np.asarray(w1, dtype=np.float32)
    w2 = np.asarray(w2, dtype=np.float32)
    w3 = np.asarray(w3, dtype=np.float32)

    nc = _get_program()
    in_maps = [stage_inputs(x[e], w1[e], w2[e], w3[e]) for e in range(E)]
    res = run_bass_kernel_spmd(nc, in_maps, list(range(E)))
    out = np.stack([res.results[e]["out"] for e in range(E)], axis=0)
    return out.astype(np.float32)
